# revision 27
# baseline (speedup 1.0000x reference)
"""TRN2 Bass kernel for nn_ODEModel (RK4 neural ODE, dense MLP vector field).

Strategy: 8-way MODEL-parallel over the 2048-dim hidden layer (W2 rows /
W3 cols sharded, 256 dims per core; W2 chunk SBUF-resident in bf16).
Every core holds the FULL batch (256), split into two halves of 128
processed in anti-phase: while half A's AllGather (partial pre-tanh sums
across the 8 cores) is in flight, the PE computes half B's matmuls, so
the collective latency and the serial tanh/RK4 tail hide under real work.

Per f-eval (per half, per core):
  h1  = relu(W1 @ y + b1)          replicated   [4096, 128] (transposed)
  h2c = relu(W2c @ h1 + b2c)       local 256    [256, 128]
  ppc = W3c @ h2c                  partial      [4, 128]
  AllGather(ppc) -> one PE matmul with stacked identities sums the 8
  partials + b3 -> tanh -> k = tanh + poly(y);  RK4 update in fp32.

Scheduling notes (from NTFF profiling):
- W1 is zero-padded to 128 contraction rows ([W1.T; b1; 0...]) so the
  stationary-weight loads are full [128,128] bf16 tiles; the y state
  lives in a [128, BS] tile whose rows 5..127 stay zero.
- The gth DMA (which waits on the AllGather) gets the Sync HWDGE queue
  to itself; yshb/inb/out DMAs ride the Act queue so neither blocks the
  other (head-of-line blocking on a DMA queue serializes the pipeline).
- h1 relu copies PSUM->SBUF are split per group across Vector+Scalar so
  neither engine's backlog gates the h2 matmul stream or the AllGather
  trigger chain.
- poly matmuls run at the end of compute (in the collective shadow);
  RK4 bookkeeping is split into rp = p1+2p2+2p3 (compute phase, frees
  the poly PSUM tile early) and rt = th1+2th2+2th3 (pockets), keeping
  the PSUM pools at exactly 8 banks.
"""
import sys

sys.path.insert(0, "/opt/trn_rl_repo")
import numpy as np
import ml_dtypes

import concourse.bass as bass
import concourse.bacc as bacc
import concourse.tile as tile
import concourse.mybir as mybir

F32 = mybir.dt.float32
BF16 = mybir.dt.bfloat16
NP_BF16 = ml_dtypes.bfloat16

N_CORES = 8
B_FULL = 256
D = 4
H1 = 4096
H2 = 2048
H2L = H2 // N_CORES   # 256 h2 dims per core
K1 = H1 // 128        # 32 contraction chunks for W2
M2 = H2L // 128       # 2 output chunks of local h2
BS = 128              # half-batch per phase
GH = 8                # h1 m-chunks per PSUM tile


def build_g(hs):
    T1 = len(hs)
    NE = 4 * T1
    nc = bacc.Bacc("TRN2", target_bir_lowering=False, debug=False,
                   num_devices=N_CORES)

    d_y0T = nc.dram_tensor("y0T", [5, B_FULL], F32, kind="ExternalInput").ap()
    d_y0f = nc.dram_tensor("y0f", [128, B_FULL], BF16,
                           kind="ExternalInput").ap()
    d_w1f = nc.dram_tensor("w1f", [128, H1], BF16, kind="ExternalInput").ap()
    d_w2l = nc.dram_tensor("w2l", [128, K1 * M2 * 128], BF16,
                           kind="ExternalInput").ap()
    d_b2 = nc.dram_tensor("b2c", [128, M2], F32, kind="ExternalInput").ap()
    d_w3l = nc.dram_tensor("w3l", [128, M2 * D], BF16,
                           kind="ExternalInput").ap()
    d_eb = nc.dram_tensor("eb", [4 * N_CORES + 1, D], BF16,
                          kind="ExternalInput").ap()
    d_wpa = nc.dram_tensor("wpa", [5, D], BF16, kind="ExternalInput").ap()
    d_wpbs = nc.dram_tensor("wpbs", [D, D], BF16, kind="ExternalInput").ap()
    d_wpbc = nc.dram_tensor("wpbc", [3, D], BF16, kind="ExternalInput").ap()
    d_out = nc.dram_tensor("out", [T1, D, B_FULL], F32,
                           kind="ExternalOutput").ap()
    rg = [list(range(N_CORES))]
    EBR = 4 * N_CORES + 1
    A = mybir.AluOpType

    with tile.TileContext(nc) as tc:
        with tc.tile_pool(name="wpool", bufs=1) as wp, \
             tc.tile_pool(name="state", bufs=1) as stp, \
             tc.tile_pool(name="act", bufs=2) as actp, \
             tc.tile_pool(name="small", bufs=3) as smp, \
             tc.tile_pool(name="ps_h1", bufs=2, space="PSUM") as ps_h1, \
             tc.tile_pool(name="ps_h2", bufs=2, space="PSUM") as ps_h2, \
             tc.tile_pool(name="ps_sm", bufs=2, space="PSUM") as ps_sm, \
             tc.tile_pool(name="dramp", bufs=2, space="DRAM") as dramp:

            w1f = wp.tile([128, H1], BF16)
            w2l = wp.tile([128, K1 * M2 * 128], BF16)
            b2 = wp.tile([128, M2], F32)
            w3l = wp.tile([128, M2 * D], BF16)
            eb = wp.tile([EBR, D], BF16)
            wpa = wp.tile([5, D], BF16)
            wpbs = wp.tile([D, D], BF16)
            wpbc = wp.tile([3, D], BF16)
            for t_, d_ in ((w1f, d_w1f), (w2l, d_w2l), (b2, d_b2),
                           (w3l, d_w3l), (eb, d_eb), (wpa, d_wpa),
                           (wpbs, d_wpbs), (wpbc, d_wpbc)):
                nc.sync.dma_start(t_[:], d_)

            # per-half persistent state
            y128, gth, h1b, h2b, ybase = {}, {}, {}, {}, {}
            poly_ref, z_ref, th_ref, rt, rp, yshb_ref, phi_ref = ({}, {}, {}, {}, {}, {}, {})
            for X in range(2):
                sl = slice(X * BS, (X + 1) * BS)
                yt = stp.tile([128, BS], BF16, name=f"y128_{X}",
                              tag=f"y128_{X}")
                nc.sync.dma_start(yt[:], d_y0f[:, sl])
                y128[X] = yt
                gt = stp.tile([EBR, BS], BF16, name=f"gth{X}", tag=f"gth{X}")
                nc.sync.dma_start(gt[4 * N_CORES:EBR, :], d_y0f[4:5, sl])
                gth[X] = gt
                h1b[X] = stp.tile([128, K1 * BS], BF16, name=f"h1b{X}",
                                  tag=f"h1b{X}")
                h2b[X] = stp.tile([128, M2 * BS], BF16, name=f"h2b{X}",
                                  tag=f"h2b{X}")
                yb = smp.tile([D, BS], F32, name=f"ybase{X}", tag=f"ybase{X}")
                nc.sync.dma_start(yb[:], d_y0T[0:4, sl])
                ybase[X] = yb

            def coef(j):
                t, stage = divmod(j, 4)
                h = float(hs[t])
                return t, stage, (h / 2, h / 2, h, h / 6)[stage]

            def compute(X, j):
                t, stage, c = coef(j)
                yt = y128[X]
                yshb = actp.tile([3, BS], BF16, name=f"ysh{X}",
                                 tag=f"ysh{X}")
                nc.scalar.dma_start(yshb[:], yt[1:4, :])
                yshb_ref[X] = yshb
                # h1 = relu(W1 @ y + b1), transposed layout [4096, BS]
                for g in range(K1 // GH):
                    h1ps = ps_h1.tile([128, GH * BS], F32, name=f"h1ps{X}",
                                      tag="h1ps")
                    for q in range(GH):
                        m = g * GH + q
                        nc.tensor.matmul(h1ps[:, q * BS:(q + 1) * BS],
                                         w1f[:, m * 128:(m + 1) * 128],
                                         yt[:], start=True, stop=True)
                    dst = h1b[X][:, g * GH * BS:(g + 1) * GH * BS]
                    half = GH * BS // 2
                    nc.vector.tensor_scalar_max(
                        dst[:, 0:half], h1ps[:, 0:half], 0.0)
                    nc.scalar.activation(
                        dst[:, half:], h1ps[:, half:],
                        mybir.ActivationFunctionType.Relu)
                phis = actp.tile([D, BS], BF16, name=f"phis{X}",
                                 tag=f"phis{X}")
                nc.vector.tensor_mul(phis[:], yt[0:4, :], yt[0:4, :])
                phic = actp.tile([3, BS], BF16, name=f"phic{X}",
                                 tag=f"phic{X}")
                nc.vector.tensor_mul(phic[:], yt[0:3, :], yshb[:])
                phi_ref[X] = (phis, phic)
                # h2c = relu(W2c @ h1 + b2c)
                def h2_chunk(m):
                    h2ps = ps_h2.tile([128, BS], F32, name=f"h2ps{X}",
                                      tag="h2ps")
                    for k in range(K1):
                        nc.tensor.matmul(
                            h2ps[:],
                            w2l[:, (k * M2 + m) * 128:(k * M2 + m + 1) * 128],
                            h1b[X][:, k * BS:(k + 1) * BS],
                            start=(k == 0), stop=(k == K1 - 1))
                    if m == 0:
                        nc.scalar.activation(
                            h2b[X][:, m * BS:(m + 1) * BS], h2ps[:],
                            mybir.ActivationFunctionType.Relu,
                            bias=b2[:, m:m + 1])
                    else:
                        nc.vector.tensor_scalar(
                            h2b[X][:, m * BS:(m + 1) * BS], h2ps[:],
                            b2[:, m:m + 1], 0.0, op0=A.add, op1=A.max)
                h2_chunk(0)
                h2_chunk(1)
                # partial pre-tanh sums -> AllGather
                pp_ps = ps_sm.tile([D, BS], F32, name=f"pp{X}", tag="sm")
                for m in range(M2):
                    nc.tensor.matmul(pp_ps[:], w3l[:, m * D:(m + 1) * D],
                                     h2b[X][:, m * BS:(m + 1) * BS],
                                     start=(m == 0), stop=(m == M2 - 1))
                pp_sb = smp.tile([D, BS], BF16, name=f"ppsb{X}",
                                 tag=f"ppsb{X}")
                nc.vector.tensor_copy(pp_sb[:], pp_ps[:])
                inb = dramp.tile([D, BS], BF16, name=f"inb{X}", tag=f"inb{X}")
                outb = dramp.tile([4 * N_CORES, BS], BF16, name=f"outb{X}",
                                  tag=f"outb{X}")
                nc.scalar.dma_start(inb[:], pp_sb[:], single_packet=True)
                nc.gpsimd.collective_compute(
                    "AllGather", mybir.AluOpType.bypass,
                    replica_groups=rg,
                    ins=[inb.opt()], outs=[outb.opt()])
                nc.sync.dma_start(gth[X][0:4 * N_CORES, :], outb[:],
                                  single_packet=True)

            def poly_tail(X, j):
                t, stage, c = coef(j)
                yt = y128[X]
                phis, phic = phi_ref[X]
                poly_ps = ps_sm.tile([D, BS], F32, name=f"poly{X}",
                                     tag="sm")
                nc.tensor.matmul(poly_ps[:], wpa[:], yt[0:5, :],
                                 start=True, stop=False)
                nc.tensor.matmul(poly_ps[:], wpbs[:], phis[:],
                                 start=False, stop=False)
                nc.tensor.matmul(poly_ps[:], wpbc[:], phic[:],
                                 start=False, stop=True)
                # z = ybase + c*poly (+ c*(rt+rp) at stage 3); also fold
                # this stage's poly into rp so the PSUM tile dies here
                if stage == 3:
                    zr1 = smp.tile([D, BS], F32, name=f"zr1{X}", tag=f"zr1{X}")
                    nc.vector.scalar_tensor_tensor(
                        zr1[:], rt[X][:], c, ybase[X][:],
                        op0=A.mult, op1=A.add)
                    zr = smp.tile([D, BS], F32, name=f"zr{X}", tag=f"zr{X}")
                    nc.vector.scalar_tensor_tensor(
                        zr[:], rp[X][:], c, zr1[:], op0=A.mult, op1=A.add)
                    base = zr
                else:
                    base = ybase[X]
                z = smp.tile([D, BS], F32, name=f"z{X}", tag=f"z{X}")
                nc.vector.scalar_tensor_tensor(
                    z[:], poly_ps[:], c, base[:], op0=A.mult, op1=A.add)
                if stage == 0:
                    r0 = smp.tile([D, BS], F32, name=f"rp{X}", tag=f"rp{X}")
                    nc.vector.tensor_copy(r0[:], poly_ps[:])
                    rp[X] = r0
                elif stage < 3:
                    rn = smp.tile([D, BS], F32, name=f"rp{X}", tag=f"rp{X}")
                    nc.vector.scalar_tensor_tensor(
                        rn[:], poly_ps[:], 2.0, rp[X][:],
                        op0=A.mult, op1=A.add)
                    rp[X] = rn
                poly_ref[X] = poly_ps
                z_ref[X] = z

            def pocket(X, j):
                t, stage, c = coef(j)
                spre = ps_sm.tile([D, BS], F32, name=f"spre{X}",
                                  tag="sm")
                nc.tensor.matmul(spre[:], eb[:], gth[X][:],
                                 start=True, stop=True)
                # keep the PE (and its HAM clock) busy through the
                # tanh/y-update chain: one accumulating filler group into
                # the h2ps rotation (operands are static weights)
                jk = ps_h2.tile([128, BS], F32, name="jk", tag="h2ps")
                for i in range(8):
                    nc.tensor.matmul(jk[:], w2l[:, 0:128], w2l[:, 0:BS],
                                     start=(i == 0), stop=(i == 7))
                th = smp.tile([D, BS], F32, name=f"th{X}", tag=f"th{X}")
                nc.scalar.activation(th[:], spre[:],
                                     mybir.ActivationFunctionType.Tanh)
                # critical: next eval's y (bf16, rows 0..3 of the padded tile)
                nc.vector.scalar_tensor_tensor(
                    y128[X][0:4, :], th[:], c, z_ref[X][:],
                    op0=A.mult, op1=A.add)
                # off-critical RK4 bookkeeping (th-part accumulation)
                if stage == 0:
                    th_ref[X] = th
                elif stage < 3:
                    rn = smp.tile([D, BS], F32, name=f"rt{X}", tag=f"rt{X}")
                    prev = th_ref[X] if stage == 1 else rt[X]
                    nc.vector.scalar_tensor_tensor(
                        rn[:], th[:], 2.0, prev[:], op0=A.mult, op1=A.add)
                    rt[X] = rn
                else:
                    ynew = smp.tile([D, BS], F32, name=f"ybase{X}",
                                    tag=f"ybase{X}")
                    nc.vector.scalar_tensor_tensor(
                        ynew[:], th[:], c, z_ref[X][:], op0=A.mult, op1=A.add)
                    ybase[X] = ynew
                    nc.scalar.dma_start(d_out[t, :, X * BS:(X + 1) * BS],
                                      ynew[:])

            # software pipeline: halves in anti-phase; each half's poly
            # matmuls are emitted after the OTHER half's eb matmul so they
            # fill the PE gap while that half's tanh/y-update chain runs
            compute(0, 0)
            poly_tail(0, 0)
            for j in range(NE):
                compute(1, j)
                pocket(0, j)
                poly_tail(1, j)
                if j + 1 < NE:
                    compute(0, j + 1)
                pocket(1, j)
                if j + 1 < NE:
                    poly_tail(0, j + 1)
    nc.compile()
    return nc


def prep_inputs_g(s_grid, y0, W1, b1, W2, b2, W3, b3, wpoly):
    hs = np.diff(np.asarray(s_grid, np.float64)).astype(np.float32)
    y0T = np.concatenate([np.asarray(y0, np.float32).T,
                          np.ones((1, B_FULL), np.float32)], 0)  # [5, 256]
    y0f = np.zeros((128, B_FULL), np.float32)
    y0f[0:5] = y0T
    y0f = y0f.astype(NP_BF16)
    w1f = np.zeros((128, H1), np.float32)
    w1f[0:4] = np.asarray(W1, np.float32).T
    w1f[4] = np.asarray(b1, np.float32)
    w1f = w1f.astype(NP_BF16)
    w = np.asarray(wpoly, np.float32)
    wpa = np.zeros((5, 4), np.float32)
    wpb = np.zeros((7, 4), np.float32)
    wpa[4, 0] = w[0]; wpa[0, 0] = w[1]; wpb[0, 0] = w[2]
    wpa[4, 1] = w[3]; wpa[0, 1] = w[4]; wpb[0, 1] = w[5]
    wpa[1, 1] = w[6]; wpb[1, 1] = w[7]; wpb[4, 1] = w[8]
    wpa[4, 2] = w[9]; wpa[2, 2] = w[10]; wpb[2, 2] = w[11]
    wpa[1, 2] = w[12]; wpb[1, 2] = w[13]; wpb[5, 2] = w[14]
    wpa[4, 3] = w[15]; wpa[3, 3] = w[16]; wpb[3, 3] = w[17]
    wpa[2, 3] = w[18]; wpb[2, 3] = w[19]; wpb[6, 3] = w[20]
    wpbs = wpb[0:4].astype(NP_BF16)
    wpbc = wpb[4:7].astype(NP_BF16)
    wpa = wpa.astype(NP_BF16)
    ebm = np.zeros((4 * N_CORES + 1, 4), np.float32)
    for r in range(N_CORES):
        ebm[r * 4:(r + 1) * 4, :] = np.eye(4, dtype=np.float32)
    ebm[4 * N_CORES, :] = np.asarray(b3, np.float32)
    ebm = ebm.astype(NP_BF16)
    W2a = np.asarray(W2, np.float32)
    W3a = np.asarray(W3, np.float32)
    b2v = np.asarray(b2, np.float32)
    in_maps = []
    for c in range(N_CORES):
        w2c = W2a[c * H2L:(c + 1) * H2L, :]
        blocks = w2c.T.reshape(K1, 128, M2, 128)
        w2lm = np.ascontiguousarray(
            blocks.transpose(1, 0, 2, 3).reshape(128, K1 * M2 * 128)
        ).astype(NP_BF16)
        b2c = np.ascontiguousarray(
            b2v[c * H2L:(c + 1) * H2L].reshape(M2, 128).T)
        w3c = W3a[:, c * H2L:(c + 1) * H2L]
        w3lm = np.ascontiguousarray(
            w3c.T.reshape(M2, 128, 4).transpose(1, 0, 2).reshape(128, M2 * 4)
        ).astype(NP_BF16)
        in_maps.append({
            "y0T": y0T, "y0f": y0f, "w1f": w1f,
            "w2l": w2lm, "b2c": b2c, "w3l": w3lm, "eb": ebm, "wpa": wpa,
            "wpbs": wpbs, "wpbc": wpbc,
        })
    return hs, in_maps


def assemble_g(results, y0):
    out = results[0]["out"]  # [T1, 4, 256]
    ys = np.ascontiguousarray(out.transpose(0, 2, 1))
    return np.concatenate([np.asarray(y0, np.float32)[None], ys], 0)


_CACHE = {}
_RUNNER = {}


def _run_cached(nc, key, in_maps):
    """Compile the shard_map jit once per build and reuse it across calls
    (run_bass_kernel_spmd re-traces on every invocation)."""
    import jax
    from jax.sharding import Mesh, PartitionSpec, NamedSharding
    from jax.experimental.shard_map import shard_map
    from concourse.bass2jax import (
        _bass_exec_p, install_neuronx_cc_hook, partition_id_tensor)
    if key not in _RUNNER:
        install_neuronx_cc_hook()
        pname = (nc.partition_id_tensor.name
                 if nc.partition_id_tensor else None)
        in_names, out_names, out_avals, zeros = [], [], [], []
        for alloc in nc.m.functions[0].allocations:
            if not isinstance(alloc, mybir.MemoryLocationSet):
                continue
            name = alloc.memorylocations[0].name
            if alloc.kind == "ExternalInput":
                if name != pname:
                    in_names.append(name)
            elif alloc.kind == "ExternalOutput":
                shape = tuple(alloc.tensor_shape)
                dt = mybir.dt.np(alloc.dtype)
                out_names.append(name)
                out_avals.append(jax.core.ShapedArray(shape, dt))
                zeros.append(np.zeros(shape, dt))
        if nc.dbg_addr is not None:
            in_names.append(nc.dbg_addr.name)
        all_in = list(in_names) + list(out_names)
        if pname is not None:
            all_in.append(pname)

        def _body(*args):
            ops = list(args)
            if pname is not None:
                ops.append(partition_id_tensor())
            return tuple(_bass_exec_p.bind(
                *ops, out_avals=tuple(out_avals), in_names=tuple(all_in),
                out_names=tuple(out_names), lowering_input_output_aliases=(),
                sim_require_finite=True, sim_require_nnan=True, nc=nc))

        mesh = Mesh(np.asarray(jax.devices()[:N_CORES]), ("core",))
        specs = (PartitionSpec("core"),) * (len(in_names) + len(out_names))
        fn = jax.jit(shard_map(_body, mesh=mesh, in_specs=specs,
                               out_specs=(PartitionSpec("core"),)
                               * len(out_names), check_rep=False),
                     keep_unused=True)
        _RUNNER[key] = (fn, in_names, out_names, out_avals, zeros, mesh)
    fn, in_names, out_names, out_avals, zeros, mesh = _RUNNER[key]
    import jax as _jax
    per_core = []
    for m in in_maps:
        m = dict(m)
        if nc.dbg_addr is not None:
            m[nc.dbg_addr.name] = np.zeros((1, 2), np.uint32)
        per_core.append([np.asarray(m[n]) for n in in_names])
    cat = [np.concatenate([per_core[c][i] for c in range(N_CORES)], axis=0)
           for i in range(len(in_names))]
    cat += [np.zeros((N_CORES * z.shape[0], *z.shape[1:]), z.dtype)
            for z in zeros]
    from jax.sharding import NamedSharding as _NS, PartitionSpec as _PS
    sh = _NS(mesh, _PS("core"))
    out = fn(*[_jax.device_put(a, sh) for a in cat])
    _jax.block_until_ready(out)
    return [
        {n: np.asarray(out[i]).reshape(N_CORES, *out_avals[i].shape)[c]
         for i, n in enumerate(out_names)}
        for c in range(N_CORES)
    ]


def kernel(s_grid, y0, W1, b1, W2, b2, W3, b3, wpoly):
    """Full-input, full-output entry point. Returns [T, 256, 4] float32."""
    import os
    os.environ.setdefault("NEURON_RT_RESET_CORES", "1")
    hs, in_maps = prep_inputs_g(s_grid, y0, W1, b1, W2, b2, W3, b3, wpoly)
    key = tuple(np.asarray(hs, np.float64).round(12).tolist())
    if key not in _CACHE:
        _CACHE[key] = build_g(hs)
    nc = _CACHE[key]
    results = None
    for attempt in range(3):
        try:
            results = dict(enumerate(_run_cached(nc, key, in_maps)))
            break
        except Exception:
            _RUNNER.pop(key, None)
            if attempt == 2:
                raise
    return assemble_g(results, y0).astype(np.float32)


# revision 28
# speedup vs baseline: 1.0247x; 1.0247x over previous
"""TRN2 Bass kernel for nn_ODEModel (RK4 neural ODE, dense MLP vector field).

Strategy: 8-way MODEL-parallel over the 2048-dim hidden layer (W2 rows /
W3 cols sharded, 256 dims per core; W2 chunk SBUF-resident in bf16).
Every core holds the FULL batch (256), split into two halves of 128
processed in anti-phase: while half A's AllGather (partial pre-tanh sums
across the 8 cores) is in flight, the PE computes half B's matmuls, so
the collective latency and the serial tanh/RK4 tail hide under real work.

Per f-eval (per half, per core):
  h1  = relu(W1 @ y + b1)          replicated   [4096, 128] (transposed)
  h2c = relu(W2c @ h1 + b2c)       local 256    [256, 128]
  ppc = W3c @ h2c                  partial      [4, 128]
  AllGather(ppc) -> one PE matmul with stacked identities sums the 8
  partials + b3 -> tanh -> k = tanh + poly(y);  RK4 update in fp32.

Scheduling notes (from NTFF profiling):
- W1 is zero-padded to 128 contraction rows ([W1.T; b1; 0...]) so the
  stationary-weight loads are full [128,128] bf16 tiles; the y state
  lives in a [128, BS] tile whose rows 5..127 stay zero.
- The gth DMA (which waits on the AllGather) gets the Sync HWDGE queue
  to itself; yshb/inb/out DMAs ride the Act queue so neither blocks the
  other (head-of-line blocking on a DMA queue serializes the pipeline).
- h1 relu copies PSUM->SBUF are split per group across Vector+Scalar so
  neither engine's backlog gates the h2 matmul stream or the AllGather
  trigger chain.
- poly matmuls run at the end of compute (in the collective shadow);
  RK4 bookkeeping is split into rp = p1+2p2+2p3 (compute phase, frees
  the poly PSUM tile early) and rt = th1+2th2+2th3 (pockets), keeping
  the PSUM pools at exactly 8 banks.
"""
import sys

sys.path.insert(0, "/opt/trn_rl_repo")
import numpy as np
import ml_dtypes

import concourse.bass as bass
import concourse.bacc as bacc
import concourse.tile as tile
import concourse.mybir as mybir

F32 = mybir.dt.float32
BF16 = mybir.dt.bfloat16
NP_BF16 = ml_dtypes.bfloat16

N_CORES = 8
B_FULL = 256
D = 4
H1 = 4096
H2 = 2048
H2L = H2 // N_CORES   # 256 h2 dims per core
K1 = H1 // 128        # 32 contraction chunks for W2
M2 = H2L // 128       # 2 output chunks of local h2
BS = 128              # half-batch per phase
GH = 8                # h1 m-chunks per PSUM tile


def build_g(hs):
    T1 = len(hs)
    NE = 4 * T1
    nc = bacc.Bacc("TRN2", target_bir_lowering=False, debug=False,
                   num_devices=N_CORES)

    d_y0T = nc.dram_tensor("y0T", [5, B_FULL], F32, kind="ExternalInput").ap()
    d_y0f = nc.dram_tensor("y0f", [128, B_FULL], BF16,
                           kind="ExternalInput").ap()
    d_w1f = nc.dram_tensor("w1f", [128, H1], BF16, kind="ExternalInput").ap()
    d_w2l = nc.dram_tensor("w2l", [128, K1 * M2 * 128], BF16,
                           kind="ExternalInput").ap()
    d_b2 = nc.dram_tensor("b2c", [128, M2], F32, kind="ExternalInput").ap()
    d_w3l = nc.dram_tensor("w3l", [128, M2 * D], BF16,
                           kind="ExternalInput").ap()
    d_eb = nc.dram_tensor("eb", [4 * N_CORES + 1, D], BF16,
                          kind="ExternalInput").ap()
    d_wpa = nc.dram_tensor("wpa", [5, D], BF16, kind="ExternalInput").ap()
    d_wpbs = nc.dram_tensor("wpbs", [D, D], BF16, kind="ExternalInput").ap()
    d_wpbc = nc.dram_tensor("wpbc", [3, D], BF16, kind="ExternalInput").ap()
    d_out = nc.dram_tensor("out", [T1, D, B_FULL], F32,
                           kind="ExternalOutput").ap()
    rg = [list(range(N_CORES))]
    EBR = 4 * N_CORES + 1
    A = mybir.AluOpType

    with tile.TileContext(nc) as tc:
        with tc.tile_pool(name="wpool", bufs=1) as wp, \
             tc.tile_pool(name="state", bufs=1) as stp, \
             tc.tile_pool(name="act", bufs=2) as actp, \
             tc.tile_pool(name="small", bufs=3) as smp, \
             tc.tile_pool(name="ps_h1", bufs=2, space="PSUM") as ps_h1, \
             tc.tile_pool(name="ps_h2", bufs=2, space="PSUM") as ps_h2, \
             tc.tile_pool(name="ps_sm", bufs=2, space="PSUM") as ps_sm, \
             tc.tile_pool(name="dramp", bufs=2, space="DRAM") as dramp:

            w1f = wp.tile([128, H1], BF16)
            w2l = wp.tile([128, K1 * M2 * 128], BF16)
            b2 = wp.tile([128, M2], F32)
            w3l = wp.tile([128, M2 * D], BF16)
            eb = wp.tile([EBR, D], BF16)
            wpa = wp.tile([5, D], BF16)
            wpbs = wp.tile([D, D], BF16)
            wpbc = wp.tile([3, D], BF16)
            for t_, d_ in ((w1f, d_w1f), (w2l, d_w2l), (b2, d_b2),
                           (w3l, d_w3l), (eb, d_eb), (wpa, d_wpa),
                           (wpbs, d_wpbs), (wpbc, d_wpbc)):
                nc.sync.dma_start(t_[:], d_)

            # per-half persistent state
            y128, gth, h1b, h2b, ybase = {}, {}, {}, {}, {}
            poly_ref, z_ref, th_ref, rt, rp, yshb_ref, phi_ref = ({}, {}, {}, {}, {}, {}, {})
            for X in range(2):
                sl = slice(X * BS, (X + 1) * BS)
                yt = stp.tile([128, BS], BF16, name=f"y128_{X}",
                              tag=f"y128_{X}")
                nc.sync.dma_start(yt[:], d_y0f[:, sl])
                y128[X] = yt
                gt = stp.tile([EBR, BS], BF16, name=f"gth{X}", tag=f"gth{X}")
                nc.sync.dma_start(gt[4 * N_CORES:EBR, :], d_y0f[4:5, sl])
                gth[X] = gt
                h1b[X] = stp.tile([128, K1 * BS], BF16, name=f"h1b{X}",
                                  tag=f"h1b{X}")
                h2b[X] = stp.tile([128, M2 * BS], BF16, name=f"h2b{X}",
                                  tag=f"h2b{X}")
                yb = smp.tile([D, BS], F32, name=f"ybase{X}", tag=f"ybase{X}")
                nc.sync.dma_start(yb[:], d_y0T[0:4, sl])
                ybase[X] = yb

            def coef(j):
                t, stage = divmod(j, 4)
                h = float(hs[t])
                return t, stage, (h / 2, h / 2, h, h / 6)[stage]

            def compute(X, j):
                t, stage, c = coef(j)
                yt = y128[X]
                yshb = actp.tile([3, BS], BF16, name=f"ysh{X}",
                                 tag=f"ysh{X}")
                nc.scalar.dma_start(yshb[:], yt[1:4, :])
                yshb_ref[X] = yshb
                # h1 = relu(W1 @ y + b1), transposed layout [4096, BS]
                for g in range(K1 // GH):
                    h1ps = ps_h1.tile([128, GH * BS], F32, name=f"h1ps{X}",
                                      tag="h1ps")
                    for q in range(GH):
                        m = g * GH + q
                        nc.tensor.matmul(h1ps[:, q * BS:(q + 1) * BS],
                                         w1f[:, m * 128:(m + 1) * 128],
                                         yt[:], start=True, stop=True)
                    dst = h1b[X][:, g * GH * BS:(g + 1) * GH * BS]
                    half = GH * BS // 2
                    nc.vector.tensor_scalar_max(
                        dst[:, 0:half], h1ps[:, 0:half], 0.0)
                    nc.scalar.activation(
                        dst[:, half:], h1ps[:, half:],
                        mybir.ActivationFunctionType.Relu)
                phis = actp.tile([D, BS], BF16, name=f"phis{X}",
                                 tag=f"phis{X}")
                nc.vector.tensor_mul(phis[:], yt[0:4, :], yt[0:4, :])
                phic = actp.tile([3, BS], BF16, name=f"phic{X}",
                                 tag=f"phic{X}")
                nc.vector.tensor_mul(phic[:], yt[0:3, :], yshb[:])
                phi_ref[X] = (phis, phic)
                # h2c = relu(W2c @ h1 + b2c)
                def h2_chunk(m):
                    h2ps = ps_h2.tile([128, BS], F32, name=f"h2ps{X}",
                                      tag="h2ps")
                    for k in range(K1):
                        nc.tensor.matmul(
                            h2ps[:],
                            w2l[:, (k * M2 + m) * 128:(k * M2 + m + 1) * 128],
                            h1b[X][:, k * BS:(k + 1) * BS],
                            start=(k == 0), stop=(k == K1 - 1))
                    if m == 0:
                        nc.scalar.activation(
                            h2b[X][:, m * BS:(m + 1) * BS], h2ps[:],
                            mybir.ActivationFunctionType.Relu,
                            bias=b2[:, m:m + 1])
                    else:
                        nc.vector.tensor_scalar(
                            h2b[X][:, m * BS:(m + 1) * BS], h2ps[:],
                            b2[:, m:m + 1], 0.0, op0=A.add, op1=A.max)
                h2_chunk(0)
                h2_chunk(1)
                # partial pre-tanh sums -> AllGather
                pp_ps = ps_sm.tile([D, BS], F32, name=f"pp{X}", tag="sm")
                for m in range(M2):
                    nc.tensor.matmul(pp_ps[:], w3l[:, m * D:(m + 1) * D],
                                     h2b[X][:, m * BS:(m + 1) * BS],
                                     start=(m == 0), stop=(m == M2 - 1))
                pp_sb = smp.tile([D, BS], BF16, name=f"ppsb{X}",
                                 tag=f"ppsb{X}")
                nc.vector.tensor_copy(pp_sb[:], pp_ps[:])
                inb = dramp.tile([D, BS], BF16, name=f"inb{X}", tag=f"inb{X}")
                outb = dramp.tile([4 * N_CORES, BS], BF16, name=f"outb{X}",
                                  tag=f"outb{X}")
                nc.scalar.dma_start(inb[:], pp_sb[:], single_packet=True)
                nc.gpsimd.collective_compute(
                    "AllGather", mybir.AluOpType.bypass,
                    replica_groups=rg,
                    ins=[inb.opt()], outs=[outb.opt()])
                nc.sync.dma_start(gth[X][0:4 * N_CORES, :], outb[:],
                                  single_packet=True)

            def poly_tail(X, j):
                t, stage, c = coef(j)
                yt = y128[X]
                phis, phic = phi_ref[X]
                poly_ps = ps_sm.tile([D, BS], F32, name=f"poly{X}",
                                     tag="sm")
                nc.tensor.matmul(poly_ps[:], wpa[:], yt[0:5, :],
                                 start=True, stop=False)
                nc.tensor.matmul(poly_ps[:], wpbs[:], phis[:],
                                 start=False, stop=False)
                nc.tensor.matmul(poly_ps[:], wpbc[:], phic[:],
                                 start=False, stop=True)
                # z = ybase + c*poly (+ c*(rt+rp) at stage 3); also fold
                # this stage's poly into rp so the PSUM tile dies here
                if stage == 3:
                    zr1 = smp.tile([D, BS], F32, name=f"zr1{X}", tag=f"zr1{X}")
                    nc.vector.scalar_tensor_tensor(
                        zr1[:], rt[X][:], c, ybase[X][:],
                        op0=A.mult, op1=A.add)
                    zr = smp.tile([D, BS], F32, name=f"zr{X}", tag=f"zr{X}")
                    nc.vector.scalar_tensor_tensor(
                        zr[:], rp[X][:], c, zr1[:], op0=A.mult, op1=A.add)
                    base = zr
                else:
                    base = ybase[X]
                z = smp.tile([D, BS], F32, name=f"z{X}", tag=f"z{X}")
                nc.vector.scalar_tensor_tensor(
                    z[:], poly_ps[:], c, base[:], op0=A.mult, op1=A.add)
                if stage == 0:
                    r0 = smp.tile([D, BS], F32, name=f"rp{X}", tag=f"rp{X}")
                    nc.vector.tensor_copy(r0[:], poly_ps[:])
                    rp[X] = r0
                elif stage < 3:
                    rn = smp.tile([D, BS], F32, name=f"rp{X}", tag=f"rp{X}")
                    nc.vector.scalar_tensor_tensor(
                        rn[:], poly_ps[:], 2.0, rp[X][:],
                        op0=A.mult, op1=A.add)
                    rp[X] = rn
                poly_ref[X] = poly_ps
                z_ref[X] = z

            def pocket(X, j):
                t, stage, c = coef(j)
                spre = ps_sm.tile([D, BS], F32, name=f"spre{X}",
                                  tag="sm")
                nc.tensor.matmul(spre[:], eb[:], gth[X][:],
                                 start=True, stop=True)
                th = smp.tile([D, BS], F32, name=f"th{X}", tag=f"th{X}")
                nc.scalar.activation(th[:], spre[:],
                                     mybir.ActivationFunctionType.Tanh)
                # critical: next eval's y (bf16, rows 0..3 of the padded tile)
                nc.vector.scalar_tensor_tensor(
                    y128[X][0:4, :], th[:], c, z_ref[X][:],
                    op0=A.mult, op1=A.add)
                # off-critical RK4 bookkeeping (th-part accumulation)
                if stage == 0:
                    th_ref[X] = th
                elif stage < 3:
                    rn = smp.tile([D, BS], F32, name=f"rt{X}", tag=f"rt{X}")
                    prev = th_ref[X] if stage == 1 else rt[X]
                    nc.vector.scalar_tensor_tensor(
                        rn[:], th[:], 2.0, prev[:], op0=A.mult, op1=A.add)
                    rt[X] = rn
                else:
                    ynew = smp.tile([D, BS], F32, name=f"ybase{X}",
                                    tag=f"ybase{X}")
                    nc.vector.scalar_tensor_tensor(
                        ynew[:], th[:], c, z_ref[X][:], op0=A.mult, op1=A.add)
                    ybase[X] = ynew
                    nc.scalar.dma_start(d_out[t, :, X * BS:(X + 1) * BS],
                                      ynew[:])

            # software pipeline: halves in anti-phase; each half's poly
            # matmuls are emitted after the OTHER half's eb matmul so they
            # fill the PE gap while that half's tanh/y-update chain runs
            compute(0, 0)
            poly_tail(0, 0)
            for j in range(NE):
                compute(1, j)
                pocket(0, j)
                poly_tail(1, j)
                if j + 1 < NE:
                    compute(0, j + 1)
                pocket(1, j)
                if j + 1 < NE:
                    poly_tail(0, j + 1)
    nc.compile()
    return nc


def prep_inputs_g(s_grid, y0, W1, b1, W2, b2, W3, b3, wpoly):
    hs = np.diff(np.asarray(s_grid, np.float64)).astype(np.float32)
    y0T = np.concatenate([np.asarray(y0, np.float32).T,
                          np.ones((1, B_FULL), np.float32)], 0)  # [5, 256]
    y0f = np.zeros((128, B_FULL), np.float32)
    y0f[0:5] = y0T
    y0f = y0f.astype(NP_BF16)
    w1f = np.zeros((128, H1), np.float32)
    w1f[0:4] = np.asarray(W1, np.float32).T
    w1f[4] = np.asarray(b1, np.float32)
    w1f = w1f.astype(NP_BF16)
    w = np.asarray(wpoly, np.float32)
    wpa = np.zeros((5, 4), np.float32)
    wpb = np.zeros((7, 4), np.float32)
    wpa[4, 0] = w[0]; wpa[0, 0] = w[1]; wpb[0, 0] = w[2]
    wpa[4, 1] = w[3]; wpa[0, 1] = w[4]; wpb[0, 1] = w[5]
    wpa[1, 1] = w[6]; wpb[1, 1] = w[7]; wpb[4, 1] = w[8]
    wpa[4, 2] = w[9]; wpa[2, 2] = w[10]; wpb[2, 2] = w[11]
    wpa[1, 2] = w[12]; wpb[1, 2] = w[13]; wpb[5, 2] = w[14]
    wpa[4, 3] = w[15]; wpa[3, 3] = w[16]; wpb[3, 3] = w[17]
    wpa[2, 3] = w[18]; wpb[2, 3] = w[19]; wpb[6, 3] = w[20]
    wpbs = wpb[0:4].astype(NP_BF16)
    wpbc = wpb[4:7].astype(NP_BF16)
    wpa = wpa.astype(NP_BF16)
    ebm = np.zeros((4 * N_CORES + 1, 4), np.float32)
    for r in range(N_CORES):
        ebm[r * 4:(r + 1) * 4, :] = np.eye(4, dtype=np.float32)
    ebm[4 * N_CORES, :] = np.asarray(b3, np.float32)
    ebm = ebm.astype(NP_BF16)
    W2a = np.asarray(W2, np.float32)
    W3a = np.asarray(W3, np.float32)
    b2v = np.asarray(b2, np.float32)
    in_maps = []
    for c in range(N_CORES):
        w2c = W2a[c * H2L:(c + 1) * H2L, :]
        blocks = w2c.T.reshape(K1, 128, M2, 128)
        w2lm = np.ascontiguousarray(
            blocks.transpose(1, 0, 2, 3).reshape(128, K1 * M2 * 128)
        ).astype(NP_BF16)
        b2c = np.ascontiguousarray(
            b2v[c * H2L:(c + 1) * H2L].reshape(M2, 128).T)
        w3c = W3a[:, c * H2L:(c + 1) * H2L]
        w3lm = np.ascontiguousarray(
            w3c.T.reshape(M2, 128, 4).transpose(1, 0, 2).reshape(128, M2 * 4)
        ).astype(NP_BF16)
        in_maps.append({
            "y0T": y0T, "y0f": y0f, "w1f": w1f,
            "w2l": w2lm, "b2c": b2c, "w3l": w3lm, "eb": ebm, "wpa": wpa,
            "wpbs": wpbs, "wpbc": wpbc,
        })
    return hs, in_maps


def assemble_g(results, y0):
    out = results[0]["out"]  # [T1, 4, 256]
    ys = np.ascontiguousarray(out.transpose(0, 2, 1))
    return np.concatenate([np.asarray(y0, np.float32)[None], ys], 0)


_CACHE = {}
_RUNNER = {}


def _run_cached(nc, key, in_maps):
    """Compile the shard_map jit once per build and reuse it across calls
    (run_bass_kernel_spmd re-traces on every invocation)."""
    import jax
    from jax.sharding import Mesh, PartitionSpec, NamedSharding
    from jax.experimental.shard_map import shard_map
    from concourse.bass2jax import (
        _bass_exec_p, install_neuronx_cc_hook, partition_id_tensor)
    if key not in _RUNNER:
        install_neuronx_cc_hook()
        pname = (nc.partition_id_tensor.name
                 if nc.partition_id_tensor else None)
        in_names, out_names, out_avals, zeros = [], [], [], []
        for alloc in nc.m.functions[0].allocations:
            if not isinstance(alloc, mybir.MemoryLocationSet):
                continue
            name = alloc.memorylocations[0].name
            if alloc.kind == "ExternalInput":
                if name != pname:
                    in_names.append(name)
            elif alloc.kind == "ExternalOutput":
                shape = tuple(alloc.tensor_shape)
                dt = mybir.dt.np(alloc.dtype)
                out_names.append(name)
                out_avals.append(jax.core.ShapedArray(shape, dt))
                zeros.append(np.zeros(shape, dt))
        if nc.dbg_addr is not None:
            in_names.append(nc.dbg_addr.name)
        all_in = list(in_names) + list(out_names)
        if pname is not None:
            all_in.append(pname)

        def _body(*args):
            ops = list(args)
            if pname is not None:
                ops.append(partition_id_tensor())
            return tuple(_bass_exec_p.bind(
                *ops, out_avals=tuple(out_avals), in_names=tuple(all_in),
                out_names=tuple(out_names), lowering_input_output_aliases=(),
                sim_require_finite=True, sim_require_nnan=True, nc=nc))

        mesh = Mesh(np.asarray(jax.devices()[:N_CORES]), ("core",))
        specs = (PartitionSpec("core"),) * (len(in_names) + len(out_names))
        fn = jax.jit(shard_map(_body, mesh=mesh, in_specs=specs,
                               out_specs=(PartitionSpec("core"),)
                               * len(out_names), check_rep=False),
                     keep_unused=True)
        _RUNNER[key] = (fn, in_names, out_names, out_avals, zeros, mesh)
    fn, in_names, out_names, out_avals, zeros, mesh = _RUNNER[key]
    import jax as _jax
    per_core = []
    for m in in_maps:
        m = dict(m)
        if nc.dbg_addr is not None:
            m[nc.dbg_addr.name] = np.zeros((1, 2), np.uint32)
        per_core.append([np.asarray(m[n]) for n in in_names])
    cat = [np.concatenate([per_core[c][i] for c in range(N_CORES)], axis=0)
           for i in range(len(in_names))]
    cat += [np.zeros((N_CORES * z.shape[0], *z.shape[1:]), z.dtype)
            for z in zeros]
    from jax.sharding import NamedSharding as _NS, PartitionSpec as _PS
    sh = _NS(mesh, _PS("core"))
    out = fn(*[_jax.device_put(a, sh) for a in cat])
    _jax.block_until_ready(out)
    return [
        {n: np.asarray(out[i]).reshape(N_CORES, *out_avals[i].shape)[c]
         for i, n in enumerate(out_names)}
        for c in range(N_CORES)
    ]


def kernel(s_grid, y0, W1, b1, W2, b2, W3, b3, wpoly):
    """Full-input, full-output entry point. Returns [T, 256, 4] float32."""
    import os
    os.environ.setdefault("NEURON_RT_RESET_CORES", "1")
    hs, in_maps = prep_inputs_g(s_grid, y0, W1, b1, W2, b2, W3, b3, wpoly)
    key = tuple(np.asarray(hs, np.float64).round(12).tolist())
    if key not in _CACHE:
        _CACHE[key] = build_g(hs)
    nc = _CACHE[key]
    results = None
    for attempt in range(3):
        try:
            results = dict(enumerate(_run_cached(nc, key, in_maps)))
            break
        except Exception:
            _RUNNER.pop(key, None)
            if attempt == 2:
                raise
    return assemble_g(results, y0).astype(np.float32)
